# revision 3
# baseline (speedup 1.0000x reference)
"""Trainium2 Bass kernel for nn_Cross_26998164423018.

Cross-attention transformer block, B=8 batch elements data-parallel over 8
NeuronCores (one batch element per core). Per core:

  xn = LN(x) ; yn = LN(y)
  q = yn @ Wq.T ; k = xn @ Wk.T ; v = xn @ Wv.T      (num_heads=1)
  attn = sigmoid(q k^T / sqrt(C))
  out  = LN( attn @ v @ Wp.T + bp + xn )

Layout strategy (everything fully unrolled, no dynamic loops):
  - LN in natural layout [tokens, C] (per-token stats are per-partition).
  - PE transposes produce activations in [C, tokens] panels for projections.
  - q^T, k^T computed in [d, tokens] layout so the attention matmul
    A^T[m,n] = sum_d k^T[d,m] q^T[d,n] needs no further transposes (sigmoid
    is elementwise -> no softmax row coupling, so computing A^T is free).
  - out2^T[d,n] = sum_m v[m,d] A^T[m,n] accumulates over m-tiles in PSUM
    while A^T tiles are produced, flash-attention style (A^T never fully
    materialized).
  - All big matmuls run in float32r (fp32 range, 11-bit mantissa, 1 cyc/row)
    with fp32 PSUM accumulation; LayerNorm & elementwise in fp32.

Host-side folding: LN gains fold into weight columns, LN shifts fold into
per-output-channel bias vectors; attention scale folds into Wq; the final
projection bias + residual shift ride a ones-row extra K-panel of the out3
matmul.
"""

import sys

for _p in ("/opt/trn_rl_repo", "/root/.axon_site/_ro/trn_rl_repo"):
    if _p not in sys.path:
        sys.path.insert(0, _p)

import numpy as np

import concourse.bass as bass
import concourse.tile as tile
from concourse import mybir, bacc
from concourse.bass_utils import run_bass_kernel_spmd
from concourse.masks import make_identity

F32 = mybir.dt.float32
F32R = mybir.dt.float32r
AF = mybir.ActivationFunctionType

P = 128
N = 2048            # tokens per core
C = 576             # model dim
B = 8               # batch == cores
KP = 5              # ceil(C/128) channel panels (last holds 64 valid rows)
NT = 16             # 128-token tiles
NCH = 4             # 512-token chunks
CW = 512            # chunk width
DT = [128, 128, 128, 128, 64]   # output-channel tile sizes
DOF = [0, 128, 256, 384, 512]
OC = [(0, 288), (288, 288)]     # out3 free-dim chunks
EPS = 1e-5
SCALE = C ** -0.5


def _ln_stats(nc, pools, xt):
    """bn_stats over 576 = 2x288 -> (mean, rstd) [P,1] fp32 tiles."""
    stats = pools["st"].tile([P, 2, 6], F32, tag="st_stats")
    nc.vector.bn_stats(stats[:, 0, :], xt[:, 0:288])
    nc.vector.bn_stats(stats[:, 1, :], xt[:, 288:576])
    mv = pools["st"].tile([P, 2], F32, tag="st_mv")
    nc.vector.bn_aggr(mv[:], stats[:])
    rs = pools["st"].tile([P, 1], F32, tag="st_rs")
    nc.scalar.activation(rs[:], mv[:, 1:2], AF.Sqrt, bias=pools["eps"][:])
    nc.vector.reciprocal(rs[:], rs[:])
    return mv, rs


def build_program():
    nc = bacc.Bacc(None, target_bir_lowering=False)

    xd = nc.dram_tensor("x", [N, C], F32, kind="ExternalInput")
    yd = nc.dram_tensor("y", [N, C], F32, kind="ExternalInput")
    # host-prepped weights, [P, panels, C] so they DMA straight into SBUF
    wqd = nc.dram_tensor("wqT", [P, KP, C], F32R, kind="ExternalInput")
    wkd = nc.dram_tensor("wkT", [P, KP, C], F32R, kind="ExternalInput")
    wvd = nc.dram_tensor("wvT", [P, KP, C], F32R, kind="ExternalInput")
    wpd = nc.dram_tensor("wpT", [P, KP + 1, C], F32R, kind="ExternalInput")
    bqd = nc.dram_tensor("bq", [P, KP], F32, kind="ExternalInput")
    bkd = nc.dram_tensor("bk", [P, KP], F32, kind="ExternalInput")
    bvd = nc.dram_tensor("bv", [C], F32, kind="ExternalInput")
    gxd = nc.dram_tensor("gxv", [C], F32, kind="ExternalInput")
    gzd = nc.dram_tensor("gzv", [C], F32, kind="ExternalInput")
    bzd = nc.dram_tensor("bzv", [C], F32, kind="ExternalInput")
    outd = nc.dram_tensor("out", [N, C], F32, kind="ExternalOutput")

    with tile.TileContext(nc) as tc:
        with (
            tc.tile_pool(name="const", bufs=1) as const,
            tc.tile_pool(name="big", bufs=1) as big,
            tc.tile_pool(name="wA", bufs=1) as wA,
            tc.tile_pool(name="wB", bufs=1) as wB,
            tc.tile_pool(name="actT", bufs=2) as actT,
            tc.tile_pool(name="qTc", bufs=2) as qTc,
            tc.tile_pool(name="ld", bufs=3) as ldp,
            tc.tile_pool(name="st", bufs=4) as stp,
            tc.tile_pool(name="o2", bufs=1) as o2p,
            tc.tile_pool(name="att", bufs=3) as attp,
            tc.tile_pool(name="o3", bufs=2) as o3p,
            tc.tile_pool(name="rld", bufs=2) as rldp,
            tc.tile_pool(name="psR", bufs=3, space="PSUM") as psR,
            tc.tile_pool(name="psA", bufs=5, space="PSUM") as psA,
            tc.tile_pool(name="dram", bufs=1, space="DRAM") as dramp,
        ):
            # ---------------- constants ----------------
            ident = const.tile([P, P], F32, tag="ident")
            make_identity(nc, ident)
            epst = const.tile([P, 1], F32, tag="eps")
            nc.vector.memset(epst[:], EPS)
            gx_b = const.tile([P, C], F32, tag="gx_b")
            nc.sync.dma_start(gx_b[:], gxd[:].partition_broadcast(P))
            gz_b = const.tile([P, C], F32, tag="gz_b")
            nc.sync.dma_start(gz_b[:], gzd[:].partition_broadcast(P))
            bz_b = const.tile([P, C], F32, tag="bz_b")
            nc.sync.dma_start(bz_b[:], bzd[:].partition_broadcast(P))
            bv_b = const.tile([P, C], F32, tag="bv_b")
            nc.sync.dma_start(bv_b[:], bvd[:].partition_broadcast(P))
            bq_sb = const.tile([P, KP], F32, tag="bq_sb")
            nc.sync.dma_start(bq_sb[:], bqd[:])
            bk_sb = const.tile([P, KP], F32, tag="bk_sb")
            nc.sync.dma_start(bk_sb[:], bkd[:])

            pools = {"st": stp, "eps": epst}

            # ---------------- persistent activations ----------------
            kT = big.tile([P, KP, N], F32R, tag="kT")      # k^T panels
            vN = big.tile([P, NT, C], F32R, tag="v")       # v natural
            nc.vector.memset(kT[64:128, 4, :].bitcast(F32), 0.0)

            resid_dram = dramp.tile([NT, P, C], F32, tag="resid_dram")
            qT_dram = dramp.tile([NCH, P, KP, CW], F32R, tag="qT_dram")

            # ---------------- weights for x-path ----------------
            wk = wA.tile([P, KP, C], F32R, tag="wA")
            nc.sync.dma_start(wk[:], wkd[:])
            wv = wB.tile([P, KP + 1, C], F32R, tag="wB")
            nc.sync.dma_start(wv[:, :KP, :], wvd[:])

            def transpose_chunk(src_nat, dst_chunk, t):
                """PE-transpose [128,576] natural tile into panel chunk slices."""
                for kp in range(KP):
                    w = DT[kp]
                    pt = psR.tile([P, P], F32, tag="rot")
                    nc.tensor.transpose(
                        pt[:w, :], src_nat[:, DOF[kp]:DOF[kp] + w], ident[:]
                    )
                    nc.any.tensor_copy(
                        dst_chunk[:w, kp, t * P:(t + 1) * P], pt[:w, :]
                    )

            # ================= x path: kT, v, residual spill =================
            for ch in range(NCH):
                xc = actT.tile([P, KP, CW], F32R, tag="actT")
                for t in range(4):
                    it = ch * 4 + t
                    xt = ldp.tile([P, C], F32, tag="ld")
                    nc.sync.dma_start(xt[:], xd[it * P:(it + 1) * P, :])
                    mv, rs = _ln_stats(nc, pools, xt)
                    nc.vector.tensor_scalar(
                        xt[:], xt[:], mv[:, 0:1], rs[:],
                        mybir.AluOpType.subtract, mybir.AluOpType.mult,
                    )
                    transpose_chunk(xt, xc, t)
                    # residual = xhat * gamma_x (shift folded into out3 bias row)
                    nc.vector.tensor_mul(xt[:], xt[:], gx_b[:])
                    nc.sync.dma_start(resid_dram[it, :, :], xt[:])
                # k^T tiles for this chunk
                for dt in range(KP):
                    pk = psR.tile([P, CW], F32, tag="rot")
                    for kp in range(KP):
                        nc.tensor.matmul(
                            pk[:DT[dt], :],
                            wk[:, kp, DOF[dt]:DOF[dt] + DT[dt]],
                            xc[:, kp, :],
                            start=(kp == 0), stop=(kp == KP - 1),
                        )
                    nc.vector.tensor_scalar_add(
                        kT[:DT[dt], dt, ch * CW:(ch + 1) * CW],
                        pk[:DT[dt], :],
                        bk_sb[:DT[dt], dt:dt + 1],
                    )
                # v tiles for this chunk
                for t in range(4):
                    it = ch * 4 + t
                    for oc, (off, wdt) in enumerate(OC):
                        pv = psR.tile([P, CW], F32, tag="rot")
                        for kp in range(KP):
                            nc.tensor.matmul(
                                pv[:, :wdt],
                                xc[:, kp, t * P:(t + 1) * P],
                                wv[:, kp, off:off + wdt],
                                start=(kp == 0), stop=(kp == KP - 1),
                            )
                        nc.vector.tensor_add(
                            vN[:, it, off:off + wdt],
                            pv[:, :wdt],
                            bv_b[:, off:off + wdt],
                        )

            # ================= y path: qT chunks -> DRAM =================
            wq = wA.tile([P, KP, C], F32R, tag="wA")
            nc.sync.dma_start(wq[:], wqd[:])
            for ch in range(NCH):
                yc = actT.tile([P, KP, CW], F32R, tag="actT")
                for t in range(4):
                    it = ch * 4 + t
                    yt = ldp.tile([P, C], F32, tag="ld")
                    nc.sync.dma_start(yt[:], yd[it * P:(it + 1) * P, :])
                    mv, rs = _ln_stats(nc, pools, yt)
                    nc.vector.tensor_scalar(
                        yt[:], yt[:], mv[:, 0:1], rs[:],
                        mybir.AluOpType.subtract, mybir.AluOpType.mult,
                    )
                    transpose_chunk(yt, yc, t)
                qs = qTc.tile([P, KP, CW], F32R, tag="qTc")
                for dt in range(KP):
                    pq = psR.tile([P, CW], F32, tag="rot")
                    for kp in range(KP):
                        nc.tensor.matmul(
                            pq[:DT[dt], :],
                            wq[:, kp, DOF[dt]:DOF[dt] + DT[dt]],
                            yc[:, kp, :],
                            start=(kp == 0), stop=(kp == KP - 1),
                        )
                    nc.vector.tensor_scalar_add(
                        qs[:DT[dt], dt, :], pq[:DT[dt], :],
                        bq_sb[:DT[dt], dt:dt + 1],
                    )
                nc.sync.dma_start(qT_dram[ch, :, :, :], qs[:])

            # ================= attention + out-projection =================
            wp = wB.tile([P, KP + 1, C], F32R, tag="wB")
            nc.sync.dma_start(wp[:], wpd[:])
            for ch in range(NCH):
                qc = qTc.tile([P, KP, CW], F32R, tag="qTc")
                nc.sync.dma_start(qc[:], qT_dram[ch, :, :, :])
                o2t = o2p.tile([P, KP + 1, CW], F32R, tag="o2t")
                nc.vector.memset(o2t[:, KP, :].bitcast(F32), 0.0)
                nc.vector.memset(o2t[0:1, KP, :].bitcast(F32), 1.0)
                accs = [
                    psA.tile([P, CW], F32, tag="acc", name=f"acc{ch}_{dt}")
                    for dt in range(KP)
                ]
                for m in range(NT):
                    pa = psR.tile([P, CW], F32, tag="rot")
                    for kp in range(KP):
                        nc.tensor.matmul(
                            pa[:],
                            kT[:, kp, m * P:(m + 1) * P],
                            qc[:, kp, :],
                            start=(kp == 0), stop=(kp == KP - 1),
                        )
                    sg = attp.tile([P, CW], F32R, tag="sg")
                    nc.scalar.activation(sg[:], pa[:], AF.Sigmoid)
                    for dt in range(KP):
                        nc.tensor.matmul(
                            accs[dt][:DT[dt], :],
                            vN[:, m, DOF[dt]:DOF[dt] + DT[dt]],
                            sg[:],
                            start=(m == 0), stop=(m == NT - 1),
                            skip_group_check=True,
                        )
                for dt in range(KP):
                    nc.any.tensor_copy(o2t[:DT[dt], dt, :], accs[dt][:DT[dt], :])
                for t in range(4):
                    it = ch * 4 + t
                    rl = rldp.tile([P, C], F32, tag="rld")
                    nc.sync.dma_start(rl[:], resid_dram[it, :, :])
                    o3 = o3p.tile([P, C], F32, tag="o3")
                    for oc, (off, wdt) in enumerate(OC):
                        p3 = psR.tile([P, CW], F32, tag="rot")
                        for kp in range(KP + 1):
                            nc.tensor.matmul(
                                p3[:, :wdt],
                                o2t[:, kp, t * P:(t + 1) * P],
                                wp[:, kp, off:off + wdt],
                                start=(kp == 0), stop=(kp == KP),
                            )
                        nc.vector.tensor_add(
                            o3[:, off:off + wdt], p3[:, :wdt], rl[:, off:off + wdt]
                        )
                    mv, rs = _ln_stats(nc, pools, o3)
                    nc.vector.tensor_scalar(
                        o3[:], o3[:], mv[:, 0:1], rs[:],
                        mybir.AluOpType.subtract, mybir.AluOpType.mult,
                    )
                    nc.vector.tensor_mul(o3[:], o3[:], gz_b[:])
                    nc.vector.tensor_add(o3[:], o3[:], bz_b[:])
                    nc.sync.dma_start(outd[it * P:(it + 1) * P, :], o3[:])

    nc.compile()
    return nc


def _pad_panels(wt, panels):
    """[C_in, C_out] -> [P, panels, C_out] with zero-padded K rows."""
    out = np.zeros((panels * P, wt.shape[1]), np.float32)
    out[: wt.shape[0]] = wt
    return np.ascontiguousarray(
        out.reshape(panels, P, wt.shape[1]).transpose(1, 0, 2)
    )


def _prep_host(inputs):
    """Fold LN affine params + attention scale into weights/biases."""
    Wq = np.asarray(inputs["Wq"], np.float32)
    Wk = np.asarray(inputs["Wk"], np.float32)
    Wv = np.asarray(inputs["Wv"], np.float32)
    Wp = np.asarray(inputs["Wp"], np.float32)
    bp = np.asarray(inputs["bp"], np.float32)
    gx = np.asarray(inputs["gx"], np.float32)
    bx = np.asarray(inputs["bx"], np.float32)
    gy = np.asarray(inputs["gy"], np.float32)
    by = np.asarray(inputs["by"], np.float32)
    gz = np.asarray(inputs["gz"], np.float32)
    bz = np.asarray(inputs["bz"], np.float32)

    wqT = _pad_panels((SCALE * Wq * gy[None, :]).T.astype(np.float32), KP)
    wkT = _pad_panels((Wk * gx[None, :]).T.astype(np.float32), KP)
    wvT = _pad_panels((Wv * gx[None, :]).T.astype(np.float32), KP)
    # out-projection, extra ones-row panel carries (bp + bx)
    wpT = np.zeros((P, KP + 1, C), np.float32)
    wpT[:, :KP, :] = _pad_panels(Wp.T.astype(np.float32), KP)
    wpT[0, KP, :] = bp + bx

    def _bias_tile(b):
        pad = np.zeros(KP * P, np.float32)
        pad[:C] = b
        return np.ascontiguousarray(pad.reshape(KP, P).T)

    bq = _bias_tile(SCALE * (Wq @ by))
    bk = _bias_tile(Wk @ bx)
    bv = (Wv @ bx).astype(np.float32)

    return {
        "wqT": wqT, "wkT": wkT, "wvT": wvT, "wpT": wpT,
        "bq": bq, "bk": bk, "bv": bv,
        "gxv": gx, "gzv": gz, "bzv": bz,
    }


_NC = None


def _get_nc():
    global _NC
    if _NC is None:
        _NC = build_program()
    return _NC


def make_in_maps(**inputs):
    shared = _prep_host(inputs)
    x = np.asarray(inputs["x"], np.float32)
    y = np.asarray(inputs["y"], np.float32)
    return [
        {"x": np.ascontiguousarray(x[b]), "y": np.ascontiguousarray(y[b]), **shared}
        for b in range(B)
    ]


def kernel(**inputs) -> np.ndarray:
    nc = _get_nc()
    in_maps = make_in_maps(**inputs)
    res = run_bass_kernel_spmd(nc, in_maps, core_ids=list(range(B)))
    return np.stack([res.results[b]["out"] for b in range(B)]).astype(np.float32)


# revision 6
# speedup vs baseline: 1.0520x; 1.0520x over previous
"""Trainium2 Bass kernel for nn_Cross_26998164423018.

Cross-attention transformer block, B=8 batch elements data-parallel over 8
NeuronCores (one batch element per core). Per core:

  xn = LN(x) ; yn = LN(y)
  q = yn @ Wq.T ; k = xn @ Wk.T ; v = xn @ Wv.T      (num_heads=1)
  attn = sigmoid(q k^T / sqrt(C))
  out  = LN( attn @ v @ Wp.T + bp + xn )

Layout strategy (fully unrolled, no dynamic loops):
  - LN in natural layout [tokens, C] (per-token stats are per-partition).
  - PE transposes produce activation chunks in [C, tokens] panel layout.
  - q^T, k^T computed in [d, tokens] layout so the attention matmul
    A^T[m,n] = sum_d k^T[d,m] q^T[d,n] needs no further transposes (sigmoid
    is elementwise -> no softmax row coupling, so computing A^T is free).
  - out2^T[d,n] = sum_m v[m,d] A^T[m,n] accumulates over m-tiles in PSUM
    while A^T tiles stream through SBUF (never fully materialized).
  - All projection biases (from LN shift folding + bp) ride ones-row extra
    K-panels of the matmuls, so every PSUM->SBUF copy is a plain copy.
  - Big matmuls in float32r (fp32 range, 11-bit mantissa, 1 cyc/row) with
    fp32 PSUM accumulation; LayerNorm & elementwise in fp32.
  - q^T and the residual are staged through DRAM to fit SBUF.
  - Emission is software-pipelined: the next chunk's load+LN runs one chunk
    ahead, its transposes interleave into the current chunk's matmul stream,
    and each attention chunk interleaves the previous chunk's output
    projection + final LN into its own matmul stream.
"""

import sys

for _p in ("/opt/trn_rl_repo", "/root/.axon_site/_ro/trn_rl_repo"):
    if _p not in sys.path:
        sys.path.insert(0, _p)

import numpy as np

import concourse.bass as bass
import concourse.tile as tile
from concourse import mybir, bacc
from concourse.bass_utils import run_bass_kernel_spmd
from concourse.masks import make_identity

F32 = mybir.dt.float32
F32R = mybir.dt.float32r
AF = mybir.ActivationFunctionType

P = 128
N = 2048            # tokens per core
C = 576             # model dim
B = 8               # batch == cores
KP = 5              # ceil(C/128) channel panels (last holds 64 valid rows)
NT = 16             # 128-token tiles
NCH = 4             # 512-token chunks
CW = 512            # chunk width
DT = [128, 128, 128, 128, 64]   # output-channel tile sizes
DOF = [0, 128, 256, 384, 512]
OC = [(0, 288), (288, 288)]     # out3 free-dim chunks
EPS = 1e-5
SCALE = C ** -0.5


def build_program():
    nc = bacc.Bacc(None, target_bir_lowering=False)

    xd = nc.dram_tensor("x", [N, C], F32, kind="ExternalInput")
    yd = nc.dram_tensor("y", [N, C], F32, kind="ExternalInput")
    # host-prepped weights, [P, 6, C]: 5 data panels + ones/bias panel
    wqd = nc.dram_tensor("wqT", [P, KP + 1, C], F32R, kind="ExternalInput")
    wkd = nc.dram_tensor("wkT", [P, KP + 1, C], F32R, kind="ExternalInput")
    wvd = nc.dram_tensor("wvT", [P, KP + 1, C], F32R, kind="ExternalInput")
    wpd = nc.dram_tensor("wpT", [P, KP + 1, C], F32R, kind="ExternalInput")
    gxd = nc.dram_tensor("gxv", [C], F32, kind="ExternalInput")
    gzd = nc.dram_tensor("gzv", [C], F32, kind="ExternalInput")
    bzd = nc.dram_tensor("bzv", [C], F32, kind="ExternalInput")
    outd = nc.dram_tensor("out", [N, C], F32, kind="ExternalOutput")

    with tile.TileContext(nc) as tc:
        with (
            tc.tile_pool(name="const", bufs=1) as const,
            tc.tile_pool(name="big", bufs=1) as big,
            tc.tile_pool(name="wA", bufs=1) as wA,
            tc.tile_pool(name="wB", bufs=1) as wB,
            tc.tile_pool(name="actT", bufs=2) as actT,
            tc.tile_pool(name="qTc", bufs=2) as qTc,
            tc.tile_pool(name="ld", bufs=4) as ldp,
            tc.tile_pool(name="st", bufs=4) as stp,
            tc.tile_pool(name="o2", bufs=1) as o2p,
            tc.tile_pool(name="att", bufs=2) as attp,
            tc.tile_pool(name="o3", bufs=2) as o3p,
            tc.tile_pool(name="rld", bufs=2) as rldp,
            tc.tile_pool(name="psR", bufs=3, space="PSUM") as psR,
            tc.tile_pool(name="psA", bufs=5, space="PSUM") as psA,
            tc.tile_pool(name="dram", bufs=1, space="DRAM") as dramp,
        ):
            # ---------------- constants ----------------
            ident = const.tile([P, P], F32, tag="ident")
            make_identity(nc, ident)
            epst = const.tile([P, 1], F32, tag="eps")
            nc.vector.memset(epst[:], EPS)
            gx_b = const.tile([P, C], F32, tag="gx_b")
            nc.sync.dma_start(gx_b[:], gxd[:].partition_broadcast(P))
            gz_b = const.tile([P, C], F32, tag="gz_b")
            nc.sync.dma_start(gz_b[:], gzd[:].partition_broadcast(P))
            bz_b = const.tile([P, C], F32, tag="bz_b")
            nc.sync.dma_start(bz_b[:], bzd[:].partition_broadcast(P))

            # ---------------- persistent activations ----------------
            kT = big.tile([P, KP, N], F32R, tag="kT")      # k^T panels
            vN = big.tile([P, NT, C], F32R, tag="v")       # v natural
            nc.vector.memset(kT[64:128, 4, :].bitcast(F32), 0.0)

            resid_dram = dramp.tile([NT, P, C], F32, tag="resid_dram")
            qT_dram = dramp.tile([NCH, P, KP, CW], F32R, tag="qT_dram")

            def _ln_stats(xt):
                stats = stp.tile([P, 2, 6], F32, tag="st_stats")
                nc.vector.bn_stats(stats[:, 0, :], xt[:, 0:288])
                nc.vector.bn_stats(stats[:, 1, :], xt[:, 288:576])
                mv = stp.tile([P, 2], F32, tag="st_mv")
                nc.vector.bn_aggr(mv[:], stats[:])
                rs = stp.tile([P, 1], F32, tag="st_rs")
                nc.scalar.activation(rs[:], mv[:, 1:2], AF.Sqrt, bias=epst[:])
                nc.vector.reciprocal(rs[:], rs[:])
                return mv, rs

            def emit_L1(src_dram, ch, key):
                """Load + LN for the 4 token tiles of a chunk."""
                tiles = {}
                for t in range(4):
                    it = ch * 4 + t
                    xt = ldp.tile([P, C], F32, tag="ld", name=f"ld{key}{it}")
                    nc.sync.dma_start(xt[:], src_dram[it * P:(it + 1) * P, :])
                    mv, rs = _ln_stats(xt)
                    nc.vector.tensor_scalar(
                        xt[:], xt[:], mv[:, 0:1], rs[:],
                        mybir.AluOpType.subtract, mybir.AluOpType.mult,
                    )
                    tiles[t] = xt
                return tiles

            def make_L2(tiles, xc, ch, key, spill_resid):
                """Per-tile transpose-into-panels callbacks (+ residual spill)."""
                def cb(t):
                    def emit():
                        it = ch * 4 + t
                        xt = tiles.pop(t)
                        for kp in range(KP):
                            w = DT[kp]
                            pt = psR.tile([P, P], F32, tag="rot",
                                          name=f"pt{key}{it}_{kp}")
                            nc.tensor.transpose(
                                pt[:w, :], xt[:, DOF[kp]:DOF[kp] + w], ident[:]
                            )
                            nc.any.tensor_copy(
                                xc[:w, kp, t * P:(t + 1) * P], pt[:w, :]
                            )
                        if spill_resid:
                            nc.vector.tensor_mul(xt[:], xt[:], gx_b[:])
                            nc.sync.dma_start(resid_dram[it, :, :], xt[:])
                    return emit
                return [cb(t) for t in range(4)]

            def prep_chunk_panels(xc):
                nc.vector.memset(xc[:, KP, :].bitcast(F32), 0.0)
                nc.vector.memset(xc[0:1, KP, :].bitcast(F32), 1.0)

            def emit_M_x(ch, xc, fillers):
                """k^T + v matmuls for x chunk; interleave filler callbacks."""
                fi = iter(fillers)
                for dt in range(KP):
                    pk = psR.tile([P, CW], F32, tag="rot", name=f"pk{ch}_{dt}")
                    for kp in range(KP + 1):
                        nc.tensor.matmul(
                            pk[:DT[dt], :],
                            wk[:, kp, DOF[dt]:DOF[dt] + DT[dt]],
                            xc[:, kp, :],
                            start=(kp == 0), stop=(kp == KP),
                        )
                    nc.any.tensor_copy(
                        kT[:DT[dt], dt, ch * CW:(ch + 1) * CW], pk[:DT[dt], :]
                    )
                    nxt = next(fi, None)
                    if nxt is not None:
                        nxt()
                for t in range(4):
                    it = ch * 4 + t
                    for oc, (off, wdt) in enumerate(OC):
                        pv = psR.tile([P, CW], F32, tag="rot", name=f"pv{it}_{oc}")
                        for kp in range(KP + 1):
                            nc.tensor.matmul(
                                pv[:, :wdt],
                                xc[:, kp, t * P:(t + 1) * P],
                                wv[:, kp, off:off + wdt],
                                start=(kp == 0), stop=(kp == KP),
                            )
                        nc.any.tensor_copy(vN[:, it, off:off + wdt], pv[:, :wdt])
                    nxt = next(fi, None)
                    if nxt is not None:
                        nxt()
                for nxt in fi:
                    nxt()

            def emit_M_y(ch, yc, fillers):
                """q^T matmuls for y chunk -> staged to DRAM."""
                fi = iter(fillers)
                qs = qTc.tile([P, KP, CW], F32R, tag="qTc", name=f"qs{ch}")
                for dt in range(KP):
                    pq = psR.tile([P, CW], F32, tag="rot", name=f"pq{ch}_{dt}")
                    for kp in range(KP + 1):
                        nc.tensor.matmul(
                            pq[:DT[dt], :],
                            wq[:, kp, DOF[dt]:DOF[dt] + DT[dt]],
                            yc[:, kp, :],
                            start=(kp == 0), stop=(kp == KP),
                        )
                    nc.any.tensor_copy(qs[:DT[dt], dt, :], pq[:DT[dt], :])
                    nxt = next(fi, None)
                    if nxt is not None:
                        nxt()
                nc.sync.dma_start(qT_dram[ch, :, :, :], qs[:])
                for nxt in fi:
                    nxt()

            # ================= x path: kT, v, residual spill =================
            wk = wA.tile([P, KP + 1, C], F32R, tag="wA")
            nc.sync.dma_start(wk[:], wkd[:])
            wv = wB.tile([P, KP + 1, C], F32R, tag="wB")
            nc.sync.dma_start(wv[:], wvd[:])

            xcs = {}
            xcs[0] = actT.tile([P, KP + 1, CW], F32R, tag="actT", name="xc0")
            prep_chunk_panels(xcs[0])
            t0 = emit_L1(xd, 0, "x")
            for cb in make_L2(t0, xcs[0], 0, "x", True):
                cb()

            for ch in range(NCH):
                fillers = []
                if ch + 1 < NCH:
                    xcs[ch + 1] = actT.tile(
                        [P, KP + 1, CW], F32R, tag="actT", name=f"xc{ch + 1}"
                    )
                    prep_chunk_panels(xcs[ch + 1])
                    tiles = emit_L1(xd, ch + 1, "x")
                    fillers = make_L2(tiles, xcs[ch + 1], ch + 1, "x", True)
                else:
                    # last x chunk: prefetch y chunk 0 into the pipeline
                    ycs = {}
                    ycs[0] = actT.tile(
                        [P, KP + 1, CW], F32R, tag="actT", name="yc0"
                    )
                    prep_chunk_panels(ycs[0])
                    tiles = emit_L1(yd, 0, "y")
                    fillers = make_L2(tiles, ycs[0], 0, "y", False)
                emit_M_x(ch, xcs.pop(ch), fillers)

            # ================= y path: qT chunks -> DRAM =================
            wq = wA.tile([P, KP + 1, C], F32R, tag="wA")
            nc.sync.dma_start(wq[:], wqd[:])
            for ch in range(NCH):
                fillers = []
                if ch + 1 < NCH:
                    ycs[ch + 1] = actT.tile(
                        [P, KP + 1, CW], F32R, tag="actT", name=f"yc{ch + 1}"
                    )
                    prep_chunk_panels(ycs[ch + 1])
                    tiles = emit_L1(yd, ch + 1, "y")
                    fillers = make_L2(tiles, ycs[ch + 1], ch + 1, "y", False)
                emit_M_y(ch, ycs.pop(ch), fillers)

            # ================= attention + out-projection =================
            wp = wB.tile([P, KP + 1, C], F32R, tag="wB")
            nc.sync.dma_start(wp[:], wpd[:])
            o2t = o2p.tile([P, KP + 1, CW], F32R, tag="o2t")
            nc.vector.memset(o2t[:, KP, :].bitcast(F32), 0.0)
            nc.vector.memset(o2t[0:1, KP, :].bitcast(F32), 1.0)
            accs = [
                psA.tile([P, CW], F32, tag="acc", name=f"accp{dt}")
                for dt in range(KP)
            ]

            def _make_tail(ch):
                """out3 + final LN for chunk ch as 8 emit-callbacks."""
                rls = {}
                o3s = {}

                def load_rl(t):
                    rls[t] = rldp.tile([P, C], F32, tag="rld", name=f"rl{ch}_{t}")
                    nc.sync.dma_start(rls[t][:], resid_dram[ch * 4 + t, :, :])

                load_rl(0)
                load_rl(1)

                def group(t, oc):
                    def emit():
                        it = ch * 4 + t
                        off, wdt = OC[oc]
                        if oc == 0:
                            o3s[t] = o3p.tile([P, C], F32, tag="o3",
                                              name=f"o3_{it}")
                        o3 = o3s[t]
                        p3 = psR.tile([P, CW], F32, tag="rot",
                                      name=f"p3{it}_{oc}")
                        for kp in range(KP + 1):
                            nc.tensor.matmul(
                                p3[:, :wdt],
                                o2t[:, kp, t * P:(t + 1) * P],
                                wp[:, kp, off:off + wdt],
                                start=(kp == 0), stop=(kp == KP),
                            )
                        rl = rls[t]
                        nc.vector.tensor_add(
                            o3[:, off:off + wdt], p3[:, :wdt],
                            rl[:, off:off + wdt],
                        )
                        if oc == 1:
                            del rls[t]
                            if t + 2 < 4:
                                load_rl(t + 2)
                            mv, rs = _ln_stats(o3)
                            nc.vector.tensor_scalar(
                                o3[:], o3[:], mv[:, 0:1], rs[:],
                                mybir.AluOpType.subtract, mybir.AluOpType.mult,
                            )
                            nc.vector.tensor_mul(o3[:], o3[:], gz_b[:])
                            nc.vector.tensor_add(o3[:], o3[:], bz_b[:])
                            nc.sync.dma_start(
                                outd[it * P:(it + 1) * P, :], o3[:]
                            )
                            del o3s[t]
                    return emit

                return [group(t, oc) for t in range(4) for oc in range(2)]

            qcs = {}
            qcs[0] = qTc.tile([P, KP, CW], F32R, tag="qTc", name="qc0")
            nc.sync.dma_start(qcs[0][:], qT_dram[0, :, :, :])
            pending_tail = []
            for ch in range(NCH):
                qc = qcs.pop(ch)
                if ch + 1 < NCH:   # prefetch next chunk's q^T
                    qcs[ch + 1] = qTc.tile(
                        [P, KP, CW], F32R, tag="qTc", name=f"qc{ch + 1}"
                    )
                    nc.sync.dma_start(qcs[ch + 1][:], qT_dram[ch + 1, :, :, :])
                # software pipeline: emit attn(m+1) before acc(m) so the PE
                # never waits on sigmoid(m); interleave the previous chunk's
                # out-projection tail between iterations
                sgs = {}

                def attn_mm(m, ch=ch, qc=qc, sgs=sgs):
                    pa = psR.tile([P, CW], F32, tag="rot", name=f"pa{ch}_{m}")
                    for kp in range(KP):
                        nc.tensor.matmul(
                            pa[:],
                            kT[:, kp, m * P:(m + 1) * P],
                            qc[:, kp, :],
                            start=(kp == 0), stop=(kp == KP - 1),
                        )
                    sg = attp.tile([P, CW], F32R, tag="sg", name=f"sg{ch}_{m}")
                    nc.scalar.activation(sg[:], pa[:], AF.Sigmoid)
                    sgs[m] = sg

                def acc_mm(m, ch=ch, sgs=sgs):
                    sg = sgs.pop(m)
                    for dt in range(KP):
                        nc.tensor.matmul(
                            accs[dt][:DT[dt], :],
                            vN[:, m, DOF[dt]:DOF[dt] + DT[dt]],
                            sg[:],
                            start=(m == 0), stop=(m == NT - 1),
                            skip_group_check=True,
                        )

                attn_mm(0)
                for m in range(1, NT):
                    attn_mm(m)
                    acc_mm(m - 1)
                    if m % 2 == 0 and pending_tail:
                        pending_tail.pop(0)()
                acc_mm(NT - 1)
                while pending_tail:
                    pending_tail.pop(0)()

                for dt in range(KP):
                    nc.any.tensor_copy(o2t[:DT[dt], dt, :], accs[dt][:DT[dt], :])
                pending_tail = _make_tail(ch)
            while pending_tail:
                pending_tail.pop(0)()

    nc.compile()
    return nc


def _pad_panels(wt):
    """[C_in, C_out] -> [P, KP, C_out] zero-padded K rows."""
    out = np.zeros((KP * P, wt.shape[1]), np.float32)
    out[: wt.shape[0]] = wt
    return np.ascontiguousarray(
        out.reshape(KP, P, wt.shape[1]).transpose(1, 0, 2)
    )


def _with_bias_panel(wt, bias):
    """[P, KP, C] + bias row -> [P, KP+1, C] (panel KP row 0 carries bias)."""
    out = np.zeros((P, KP + 1, C), np.float32)
    out[:, :KP, :] = wt
    out[0, KP, :] = bias
    return out


def _prep_host(inputs):
    """Fold LN affine params + attention scale into weights/biases."""
    Wq = np.asarray(inputs["Wq"], np.float32)
    Wk = np.asarray(inputs["Wk"], np.float32)
    Wv = np.asarray(inputs["Wv"], np.float32)
    Wp = np.asarray(inputs["Wp"], np.float32)
    bp = np.asarray(inputs["bp"], np.float32)
    gx = np.asarray(inputs["gx"], np.float32)
    bx = np.asarray(inputs["bx"], np.float32)
    gy = np.asarray(inputs["gy"], np.float32)
    by = np.asarray(inputs["by"], np.float32)
    gz = np.asarray(inputs["gz"], np.float32)
    bz = np.asarray(inputs["bz"], np.float32)

    wqT = _with_bias_panel(
        _pad_panels((SCALE * Wq * gy[None, :]).T.astype(np.float32)),
        SCALE * (Wq @ by),
    )
    wkT = _with_bias_panel(
        _pad_panels((Wk * gx[None, :]).T.astype(np.float32)), Wk @ bx
    )
    wvT = _with_bias_panel(
        _pad_panels((Wv * gx[None, :]).T.astype(np.float32)), Wv @ bx
    )
    wpT = _with_bias_panel(_pad_panels(Wp.T.astype(np.float32)), bp + bx)

    return {
        "wqT": wqT, "wkT": wkT, "wvT": wvT, "wpT": wpT,
        "gxv": gx, "gzv": gz, "bzv": bz,
    }


_NC = None


def _get_nc():
    global _NC
    if _NC is None:
        _NC = build_program()
    return _NC


def make_in_maps(**inputs):
    shared = _prep_host(inputs)
    x = np.asarray(inputs["x"], np.float32)
    y = np.asarray(inputs["y"], np.float32)
    return [
        {"x": np.ascontiguousarray(x[b]), "y": np.ascontiguousarray(y[b]), **shared}
        for b in range(B)
    ]


def kernel(**inputs) -> np.ndarray:
    nc = _get_nc()
    in_maps = make_in_maps(**inputs)
    res = run_bass_kernel_spmd(nc, in_maps, core_ids=list(range(B)))
    return np.stack([res.results[b]["out"] for b in range(B)]).astype(np.float32)
